# revision 12
# baseline (speedup 1.0000x reference)
"""GaussianFC Trainium2 kernel.

out = relu(x @ W + bias),  W[i, o] = amp[i] * exp(-(o - mu[i])^2 / (2 sigma[i]^2))

Strategy (8 NeuronCores, out_features sharded, 1024 cols/core):
- Banded weights: sigma ~ 10 makes W effectively zero outside |o - mu| ~ 45.
  Host sorts rows by mu; each 128-col output block reads only the 256
  nearest (in mu) input rows, sliced at arbitrary (unaligned) offsets.
- z = (sc*(o - mu))^2 is quadratic in o, so each [128, 128] z tile is a
  rank-3 outer product: a K=8 bf16 matmul on PE against a fixed basis
  {o^2_hi, o^2_lo, o, 1} with hi/lo-split per-row coefficients (exact to
  ~5e-3 in z). This removes all per-tile DVE/ACT synthesis work.
- W = Exp(-z) runs as one parameter-free ACT op per 4-ktile group,
  PSUM -> SBUF bf16 (the only transcendental; ACT is the ceiling).
- Main matmuls keep W stationary (lhs) and stream x (64 moving rows,
  bf16): out^T[o, b] accumulates in PSUM; relu (+bias) on DVE/Pool;
  output leaves in SBUF-mirrored DRAM layout, host undoes the transpose.
- PE p-state: dummy matmuls fill the ~2.4us input-DMA latency window so
  real matmuls run at full clock.
"""
import numpy as np
from contextlib import ExitStack

import ml_dtypes

import concourse.bacc as bacc
import concourse.bass as bass
import concourse.mybir as mybir
import concourse.tile as tile
from concourse import bass_utils

f32 = mybir.dt.float32
bf16 = mybir.dt.bfloat16
AF = mybir.ActivationFunctionType
ALU = mybir.AluOpType
BF = ml_dtypes.bfloat16

NCORES = 8
BATCH = 64
IN_F = 8192
OUT_F = 8192
PER_CORE = OUT_F // NCORES  # 1024
NO = 128                    # output cols per block
B = PER_CORE // NO          # 8 blocks per core
KB = 256                    # band rows per block
NKT = B * 2                 # 16 k-tiles per core
GROUPS = 4                  # 4 k-tiles (2 blocks) per Exp group
NBASIS = 8                  # quadratic basis rows (7 used + 1 pad)

# ---- tuning knobs ----
NWARM_BIG = 3    # PE warmup matmuls with 512 moving rows
NWARM_SMALL = 3  # trailing warmup matmuls with 128 moving rows
GROUP_KT = (2, 6, 6, 2)  # k-tiles per Exp group (even: whole blocks)
ZBUFS = 2
WBUFS = 3
OBUFS = 2


def _build_program(has_bias):
    nc = bacc.Bacc("TRN2", target_bir_lowering=False, debug=False,
                   num_devices=NCORES)

    xt_d = nc.dram_tensor("xt", [128, NKT * BATCH], bf16,
                          kind="ExternalInput").ap()
    par_d = nc.dram_tensor("par", [NBASIS, NKT * NO + NO], bf16,
                           kind="ExternalInput").ap()
    bias_d = nc.dram_tensor("biasv", [128, B], f32,
                            kind="ExternalInput").ap()
    out_d = nc.dram_tensor("out", [128, B * BATCH], f32,
                           kind="ExternalOutput").ap()

    gk = list(GROUP_KT)
    assert sum(gk) == NKT and all(k % 2 == 0 for k in gk)
    gs = [sum(gk[:i]) for i in range(len(gk) + 1)]  # ktile offsets

    with tile.TileContext(nc) as tc, ExitStack() as ctx:
        cpool = ctx.enter_context(tc.tile_pool(name="const", bufs=1))
        wpool = ctx.enter_context(tc.tile_pool(name="wts", bufs=WBUFS))
        spool = ctx.enter_context(tc.tile_pool(name="stage", bufs=len(gk)))
        zpool = ctx.enter_context(tc.tile_pool(name="zq", bufs=ZBUFS,
                                               space="PSUM"))
        dpool = ctx.enter_context(tc.tile_pool(name="dummy", bufs=1,
                                               space="PSUM"))
        opool = ctx.enter_context(tc.tile_pool(name="acc", bufs=OBUFS,
                                               space="PSUM"))

        t_par = cpool.tile([NBASIS, NKT * NO + NO], bf16, tag="par")
        nc.sync.dma_start(t_par[:], par_d)
        t_xt = cpool.tile([128, NKT * BATCH], bf16, tag="xt")
        nc.sync.dma_start(t_xt[:], xt_d)
        t_bias = cpool.tile([128, B], f32, tag="bias")
        nc.sync.dma_start(t_bias[:], bias_d)

        basis = t_par[:, NKT * NO: NKT * NO + NO]

        # PE warmup: keep the tensor engine continuously busy through the
        # input-DMA latency window so real matmuls run at full p-state.
        t_zero = cpool.tile([2, 512], bf16, tag="zeros")
        nc.gpsimd.memset(t_zero[:], 0)
        dp = dpool.tile([128, 512], f32, tag="dp")
        for w in range(NWARM_BIG):
            nc.tensor.matmul(dp[:], t_zero[:, :128], t_zero[:],
                             start=True, stop=True)
        for w in range(NWARM_SMALL):
            nc.tensor.matmul(dp[:, :128], t_zero[:, :128], t_zero[:, :128],
                             start=True, stop=True)

        def z_group(g):
            nkt = gk[g]
            zp = zpool.tile([128, nkt * NO], f32, tag="z")
            for t in range(nkt):
                jt = gs[g] + t
                nc.tensor.matmul(zp[:, t * NO:(t + 1) * NO],
                                 t_par[:, jt * NO:(jt + 1) * NO],
                                 basis, start=True, stop=True)
            return zp

        def exp_group(g, zp):
            wt = wpool.tile([128, gk[g] * NO], bf16, tag="w")
            nc.scalar.activation(wt[:], zp[:], AF.Exp, bias=0.0, scale=-1.0)
            return wt

        def mm_group(g, wt):
            og = opool.tile([128, (gk[g] // 2) * BATCH], f32, tag="og")
            for t in range(gk[g]):
                jt = gs[g] + t
                jl = t // 2
                nc.tensor.matmul(og[:, jl * BATCH:(jl + 1) * BATCH],
                                 wt[:, t * NO:(t + 1) * NO],
                                 t_xt[:, jt * BATCH:(jt + 1) * BATCH],
                                 start=(t % 2 == 0), stop=(t % 2 == 1))
            return og

        NG_ = len(gk)

        def relu_dma_group(g, og):
            # group g covers blocks gs[g]//2 .. gs[g+1]//2
            j0, j1 = gs[g] // 2, gs[g + 1] // 2
            last = g == NG_ - 1
            sg_own = spool.tile([128, (j1 - j0) * BATCH], f32, tag="sg")
            sg = sg_own[:]
            if has_bias:
                for j in range(j0, j1):
                    jl = j - j0
                    nc.vector.tensor_scalar(sg[:, jl * BATCH:(jl + 1) * BATCH],
                                            og[:, jl * BATCH:(jl + 1) * BATCH],
                                            t_bias[:, j:j + 1], 0.0,
                                            ALU.add, ALU.max)
            else:
                nc.vector.tensor_scalar_max(sg, og[:], 0.0)
            nc.sync.dma_start(out_d[:, j0 * BATCH:j1 * BATCH], sg)

        # Interleave so PE never stalls on ACT.
        NG = len(gk)
        zps = [None] * NG
        zps[0] = z_group(0)
        zps[1] = z_group(1)
        for g in range(NG):
            wt = exp_group(g, zps[g])
            og = mm_group(g, wt)
            if g + 2 < NG:
                zps[g + 2] = z_group(g + 2)
            relu_dma_group(g, og)

    nc.compile()
    return nc


_PROG_CACHE = {}


def _prepare(x, mu, sigma, amplitude, bias):
    """Host-side packing: sort by mu, pick per-block bands, build the
    hi/lo-split quadratic coefficients and SBUF-mirrored input maps."""
    mu_f = np.asarray(mu, dtype=np.float64).ravel()
    sg_f = np.asarray(sigma, dtype=np.float64).ravel()
    am_f = np.asarray(amplitude, dtype=np.float64).ravel()
    perm = np.argsort(mu_f, kind="stable")
    mus = mu_f[perm]
    sgs = sg_f[perm]
    ams = am_f[perm]
    xp = np.ascontiguousarray(np.asarray(x, dtype=np.float32)[:, perm])
    if not np.allclose(ams, 1.0):
        xp = xp * ams[None, :].astype(np.float32)
    x_bf = xp.astype(BF)

    nblk = NCORES * B
    centers = np.arange(nblk, dtype=np.float64) * NO + NO / 2.0
    starts = np.clip(np.searchsorted(mus, centers) - KB // 2, 0, IN_F - KB)
    rows = starts[:, None] + np.arange(KB)[None, :]          # [nblk, KB]

    sc = 1.0 / (np.sqrt(2.0) * np.maximum(sgs[rows], 1e-30))  # [nblk, KB]
    v = sc * (mus[rows] - centers[:, None])
    A = sc * sc
    Bc = -2.0 * sc * v
    C = v * v

    def hilo(a):
        hi = a.astype(BF).astype(np.float64)
        lo = (a - hi).astype(BF)
        return hi.astype(BF), lo

    Ah, Al = hilo(A)
    Bh, Bl = hilo(Bc)
    Ch, Cl = hilo(C)
    # lhs rows pair with basis rows {o2h, o2h, o2l, o, o, 1, 1, 0}
    lhs = np.stack([Ah, Al, Ah, Bh, Bl, Ch, Cl,
                    np.zeros_like(Ah)], axis=1)              # [nblk, 8, KB]

    o_rel = np.arange(NO, dtype=np.float64) - NO / 2.0
    o2 = o_rel * o_rel
    r0h = o2.astype(BF).astype(np.float64)
    r0l = (o2 - r0h).astype(BF)
    basis = np.stack([r0h.astype(BF), r0h.astype(BF), r0l,
                      o_rel.astype(BF), o_rel.astype(BF),
                      np.ones(NO, BF), np.ones(NO, BF),
                      np.zeros(NO, BF)])                     # [8, NO]

    bias_v = np.asarray(bias, dtype=np.float32).ravel()
    has_bias = bool(np.any(bias_v != 0.0))

    # x gathered per block: [BATCH, nblk, KB] -> per-core xt
    xg = x_bf[:, rows]                                       # [64, nblk, 256]

    in_maps = []
    for c in range(NCORES):
        blk = slice(c * B, (c + 1) * B)
        # par: 16 lhs tiles [8, 128] + basis [8, 128]
        lh = lhs[blk].reshape(B, NBASIS, 2, NO)              # [8blk, 8, 2, 128]
        par = np.empty((NBASIS, NKT * NO + NO), dtype=BF)
        par[:, :NKT * NO] = lh.transpose(1, 0, 2, 3).reshape(NBASIS, NKT * NO)
        par[:, NKT * NO:] = basis
        # xt: [128, NKT*BATCH], col jt*64+b = x[b, rows[jg, (jt%2)*128+p]]
        xc = xg[:, blk].reshape(BATCH, B, 2, NO)             # [64, 8, 2, 128]
        xt = np.ascontiguousarray(
            xc.transpose(3, 1, 2, 0).reshape(128, NKT * BATCH))
        bm = np.ascontiguousarray(
            bias_v[c * PER_CORE:(c + 1) * PER_CORE].reshape(B, NO).T)
        in_maps.append({"xt": xt, "par": par, "biasv": bm})
    return in_maps, has_bias


def kernel(x, mu, sigma, amplitude, bias, _trace=False):
    in_maps, has_bias = _prepare(x, mu, sigma, amplitude, bias)
    if has_bias not in _PROG_CACHE:
        _PROG_CACHE[has_bias] = _build_program(has_bias)
    nc = _PROG_CACHE[has_bias]
    res = bass_utils.run_bass_kernel_spmd(nc, in_maps, list(range(NCORES)),
                                          trace=_trace)
    out = np.empty((BATCH, OUT_F), dtype=np.float32)
    for c in range(NCORES):
        # [128, B*BATCH] -> out[b, c*1024 + j*128 + p]
        arr = res.results[c]["out"].reshape(128, B, BATCH)
        out[:, c * PER_CORE:(c + 1) * PER_CORE] = \
            arr.transpose(2, 1, 0).reshape(BATCH, PER_CORE)
    if _trace:
        kernel._last = res
    return out


# revision 17
# speedup vs baseline: 1.0383x; 1.0383x over previous
"""GaussianFC Trainium2 kernel.

out = relu(x @ W + bias),  W[i, o] = amp[i] * exp(-(o - mu[i])^2 / (2 sigma[i]^2))

Strategy (8 NeuronCores, out_features sharded, 1024 cols/core):
- Banded weights: sigma ~ 10 makes W effectively zero outside |o - mu| ~ 45.
  Host sorts rows by mu; each 128-col output block reads only the 256
  nearest (in mu) input rows, sliced at arbitrary (unaligned) offsets.
- z = (sc*(o - mu))^2 is quadratic in o, so each [128, 128] z tile is a
  rank-3 outer product: a K=8 bf16 matmul on PE against a fixed basis
  {o^2_hi, o^2_lo, o, 1} with hi/lo-split per-row coefficients (exact to
  ~5e-3 in z). This removes all per-tile DVE/ACT synthesis work.
- W = Exp(-z) runs as one parameter-free ACT op per 4-ktile group,
  PSUM -> SBUF bf16 (the only transcendental; ACT is the ceiling).
- Main matmuls keep W stationary (lhs) and stream x (64 moving rows,
  bf16): out^T[o, b] accumulates in PSUM; relu (+bias) on DVE/Pool;
  output leaves in SBUF-mirrored DRAM layout, host undoes the transpose.
- PE p-state: dummy matmuls fill the ~2.4us input-DMA latency window so
  real matmuls run at full clock.
"""
import numpy as np
from contextlib import ExitStack

import ml_dtypes

import concourse.bacc as bacc
import concourse.bass as bass
import concourse.mybir as mybir
import concourse.tile as tile
from concourse import bass_utils

f32 = mybir.dt.float32
bf16 = mybir.dt.bfloat16
AF = mybir.ActivationFunctionType
ALU = mybir.AluOpType
BF = ml_dtypes.bfloat16

NCORES = 8
BATCH = 64
IN_F = 8192
OUT_F = 8192
PER_CORE = OUT_F // NCORES  # 1024
NO = 128                    # output cols per block
B = PER_CORE // NO          # 8 blocks per core
KB = 256                    # band rows per block
NKT = B * 2                 # 16 k-tiles per core
GROUPS = 4                  # 4 k-tiles (2 blocks) per Exp group
NBASIS = 8                  # quadratic basis rows (7 used + 1 pad)

# ---- tuning knobs ----
NWARM_BIG = 3    # PE warmup matmuls with 512 moving rows
NWARM_SMALL = 3  # trailing warmup matmuls with 128 moving rows
GROUP_KT = (2, 4, 6, 4)  # k-tiles per Exp group (even: whole blocks)
ZBUFS = 2
WBUFS = 3
OBUFS = 2


def _build_program(has_bias, group_kt=None, nwarm=None, last_eng='sp'):
    nc = bacc.Bacc("TRN2", target_bir_lowering=False, debug=False,
                   num_devices=NCORES)

    xt_d = nc.dram_tensor("xt", [128, NKT * BATCH], bf16,
                          kind="ExternalInput").ap()
    par_d = nc.dram_tensor("par", [NBASIS, NKT * NO + NO], bf16,
                           kind="ExternalInput").ap()
    bias_d = nc.dram_tensor("biasv", [128, B], f32,
                            kind="ExternalInput").ap()
    out_d = nc.dram_tensor("out", [128, B * BATCH], f32,
                           kind="ExternalOutput").ap()

    gk = list(group_kt or GROUP_KT)
    nw_big, nw_small = nwarm or (NWARM_BIG, NWARM_SMALL)
    assert sum(gk) == NKT and all(k % 2 == 0 for k in gk)
    gs = [sum(gk[:i]) for i in range(len(gk) + 1)]  # ktile offsets

    with tile.TileContext(nc) as tc, ExitStack() as ctx:
        cpool = ctx.enter_context(tc.tile_pool(name="const", bufs=1))
        wpool = ctx.enter_context(tc.tile_pool(name="wts", bufs=WBUFS))
        spool = ctx.enter_context(tc.tile_pool(name="stage", bufs=len(gk)))
        zpool = ctx.enter_context(tc.tile_pool(name="zq", bufs=ZBUFS,
                                               space="PSUM"))
        opool = ctx.enter_context(tc.tile_pool(name="acc", bufs=OBUFS,
                                               space="PSUM"))

        t_par_t = cpool.tile([NBASIS, NKT * NO + NO], bf16, tag="par")
        t_par = t_par_t[:]
        nc.sync.dma_start(t_par, par_d)
        t_xt_t = cpool.tile([128, NKT * BATCH], bf16, tag="xt")
        t_xt = t_xt_t[:]
        nc.sync.dma_start(t_xt, xt_d)
        t_bias_t = cpool.tile([128, B], f32, tag="bias")
        t_bias = t_bias_t[:]
        if has_bias:
            nc.sync.dma_start(t_bias, bias_d)

        basis = t_par[:, NKT * NO: NKT * NO + NO]


        def z_group(g):
            nkt = gk[g]
            zp = zpool.tile([128, nkt * NO], f32, tag="z")
            for t in range(nkt):
                jt = gs[g] + t
                nc.tensor.matmul(zp[:, t * NO:(t + 1) * NO],
                                 t_par[:, jt * NO:(jt + 1) * NO],
                                 basis, start=True, stop=True)
            return zp

        def exp_group(g, zp):
            wt = wpool.tile([128, gk[g] * NO], bf16, tag="w")
            nc.scalar.activation(wt[:], zp[:], AF.Exp, bias=0.0, scale=-1.0)
            return wt

        def mm_group(g, wt):
            og = opool.tile([128, (gk[g] // 2) * BATCH], f32, tag="og")
            for t in range(gk[g]):
                jt = gs[g] + t
                jl = t // 2
                nc.tensor.matmul(og[:, jl * BATCH:(jl + 1) * BATCH],
                                 wt[:, t * NO:(t + 1) * NO],
                                 t_xt[:, jt * BATCH:(jt + 1) * BATCH],
                                 start=(t % 2 == 0), stop=(t % 2 == 1))
            return og

        NG_ = len(gk)

        def relu_dma_group(g, og):
            # group g covers blocks gs[g]//2 .. gs[g+1]//2
            j0, j1 = gs[g] // 2, gs[g + 1] // 2
            last = g == NG_ - 1
            sg_own = spool.tile([128, (j1 - j0) * BATCH], f32, tag="sg")
            sg = sg_own[:]
            if has_bias:
                for j in range(j0, j1):
                    jl = j - j0
                    nc.vector.tensor_scalar(sg[:, jl * BATCH:(jl + 1) * BATCH],
                                            og[:, jl * BATCH:(jl + 1) * BATCH],
                                            t_bias[:, j:j + 1], 0.0,
                                            ALU.add, ALU.max)
            else:
                if last and last_eng == 'act':
                    nc.scalar.activation(sg, og[:], AF.Relu)
                else:
                    nc.vector.tensor_scalar_max(sg, og[:], 0.0)
            eng = nc.scalar if (last and last_eng == 'act') else nc.sync
            eng.dma_start(out_d[:, j0 * BATCH:j1 * BATCH], sg)

        # Interleave so PE never stalls on ACT.
        NG = len(gk)
        zps = [None] * NG
        zps[0] = z_group(0)
        zps[1] = z_group(1)
        for g in range(NG):
            wt = exp_group(g, zps[g])
            if g + 2 < NG:
                zps[g + 2] = z_group(g + 2)
            og = mm_group(g, wt)
            relu_dma_group(g, og)

    nc.compile()
    return nc


_PROG_CACHE = {}


def _prepare(x, mu, sigma, amplitude, bias):
    """Host-side packing: sort by mu, pick per-block bands, build the
    hi/lo-split quadratic coefficients and SBUF-mirrored input maps."""
    mu_f = np.asarray(mu, dtype=np.float64).ravel()
    sg_f = np.asarray(sigma, dtype=np.float64).ravel()
    am_f = np.asarray(amplitude, dtype=np.float64).ravel()
    perm = np.argsort(mu_f, kind="stable")
    mus = mu_f[perm]
    sgs = sg_f[perm]
    ams = am_f[perm]
    xp = np.ascontiguousarray(np.asarray(x, dtype=np.float32)[:, perm])
    if not np.allclose(ams, 1.0):
        xp = xp * ams[None, :].astype(np.float32)
    x_bf = xp.astype(BF)

    nblk = NCORES * B
    centers = np.arange(nblk, dtype=np.float64) * NO + NO / 2.0
    starts = np.clip(np.searchsorted(mus, centers) - KB // 2, 0, IN_F - KB)
    rows = starts[:, None] + np.arange(KB)[None, :]          # [nblk, KB]

    sc = 1.0 / (np.sqrt(2.0) * np.maximum(sgs[rows], 1e-30))  # [nblk, KB]
    v = sc * (mus[rows] - centers[:, None])
    A = sc * sc
    Bc = -2.0 * sc * v
    C = v * v

    def hilo(a):
        hi = a.astype(BF).astype(np.float64)
        lo = (a - hi).astype(BF)
        return hi.astype(BF), lo

    Ah, Al = hilo(A)
    Bh, Bl = hilo(Bc)
    Ch, Cl = hilo(C)
    # lhs rows pair with basis rows {o2h, o2h, o2l, o, o, 1, 1, 0}
    lhs = np.stack([Ah, Al, Ah, Bh, Bl, Ch, Cl,
                    np.zeros_like(Ah)], axis=1)              # [nblk, 8, KB]

    o_rel = np.arange(NO, dtype=np.float64) - NO / 2.0
    o2 = o_rel * o_rel
    r0h = o2.astype(BF).astype(np.float64)
    r0l = (o2 - r0h).astype(BF)
    basis = np.stack([r0h.astype(BF), r0h.astype(BF), r0l,
                      o_rel.astype(BF), o_rel.astype(BF),
                      np.ones(NO, BF), np.ones(NO, BF),
                      np.zeros(NO, BF)])                     # [8, NO]

    bias_v = np.asarray(bias, dtype=np.float32).ravel()
    has_bias = bool(np.any(bias_v != 0.0))

    # x gathered per block: [BATCH, nblk, KB] -> per-core xt
    xg = x_bf[:, rows]                                       # [64, nblk, 256]

    in_maps = []
    for c in range(NCORES):
        blk = slice(c * B, (c + 1) * B)
        # par: 16 lhs tiles [8, 128] + basis [8, 128]
        lh = lhs[blk].reshape(B, NBASIS, 2, NO)              # [8blk, 8, 2, 128]
        par = np.empty((NBASIS, NKT * NO + NO), dtype=BF)
        par[:, :NKT * NO] = lh.transpose(1, 0, 2, 3).reshape(NBASIS, NKT * NO)
        par[:, NKT * NO:] = basis
        # xt: [128, NKT*BATCH], col jt*64+b = x[b, rows[jg, (jt%2)*128+p]]
        xc = xg[:, blk].reshape(BATCH, B, 2, NO)             # [64, 8, 2, 128]
        xt = np.ascontiguousarray(
            xc.transpose(3, 1, 2, 0).reshape(128, NKT * BATCH))
        bm = np.ascontiguousarray(
            bias_v[c * PER_CORE:(c + 1) * PER_CORE].reshape(B, NO).T)
        in_maps.append({"xt": xt, "par": par, "biasv": bm})
    return in_maps, has_bias


def kernel(x, mu, sigma, amplitude, bias, _trace=False):
    in_maps, has_bias = _prepare(x, mu, sigma, amplitude, bias)
    if has_bias not in _PROG_CACHE:
        _PROG_CACHE[has_bias] = _build_program(has_bias)
    nc = _PROG_CACHE[has_bias]
    res = bass_utils.run_bass_kernel_spmd(nc, in_maps, list(range(NCORES)),
                                          trace=_trace)
    out = np.empty((BATCH, OUT_F), dtype=np.float32)
    for c in range(NCORES):
        # [128, B*BATCH] -> out[b, c*1024 + j*128 + p]
        arr = res.results[c]["out"].reshape(128, B, BATCH)
        out[:, c * PER_CORE:(c + 1) * PER_CORE] = \
            arr.transpose(2, 1, 0).reshape(BATCH, PER_CORE)
    if _trace:
        kernel._last = res
    return out


# revision 20
# speedup vs baseline: 1.0524x; 1.0136x over previous
"""GaussianFC Trainium2 kernel.

out = relu(x @ W + bias),  W[i, o] = amp[i] * exp(-(o - mu[i])^2 / (2 sigma[i]^2))

Strategy (8 NeuronCores, out_features sharded, 1024 cols/core):
- Banded weights: sigma ~ 10 makes W effectively zero outside |o - mu| ~ 45.
  Host sorts rows by mu; each 128-col output block reads the 256 nearest
  (in mu) input rows at arbitrary unaligned offsets.
- z = (sc*(o - mu))^2 is quadratic in the column index, so z tiles are
  rank-3 outer products: K=8 bf16 matmuls on PE against a fixed basis
  {f^2_hi, f^2_lo, f, 1} with hi/lo-split per-partition coefficients
  (z error ~5e-3). No DVE/ACT synthesis work at all.
- Each block's 256-row band is split: the central 128 rows get the full
  128-col window; the 128 outer rows (64 left + 64 right) share one
  64-col window (left rows use the left-half columns, right rows the
  right half — the per-partition quadratic centers absorb the shift).
  That cuts synthesized area from 256 to 192 cols/block. The outer-row
  matmuls run on partition ranges [0:64) / [64:128) with separate PSUM
  stop flags.
- W = Exp(-z) runs as one parameter-free ACT op per group (PSUM -> SBUF
  bf16); the serial Exp chain is the kernel's ceiling.
- Main matmuls keep W stationary (lhs) and stream x (64 moving rows,
  bf16): out^T[o, b] accumulates in PSUM; relu (+bias) on DVE; outputs
  leave in SBUF-mirrored DRAM layout, host undoes the transpose.
"""
import numpy as np
from contextlib import ExitStack

import ml_dtypes

import concourse.bacc as bacc
import concourse.bass as bass
import concourse.mybir as mybir
import concourse.tile as tile
from concourse import bass_utils

f32 = mybir.dt.float32
bf16 = mybir.dt.bfloat16
AF = mybir.ActivationFunctionType
ALU = mybir.AluOpType
BF = ml_dtypes.bfloat16

NCORES = 8
BATCH = 64
IN_F = 8192
OUT_F = 8192
PER_CORE = OUT_F // NCORES  # 1024
NO = 128                    # output cols per block
B = PER_CORE // NO          # 8 blocks per core
KB = 256                    # band rows per block (128 core + 128 outer)
NBASIS = 8                  # quadratic basis rows (7 used + 1 pad)
WCOLS = NO + NO // 2        # 192 synthesized cols per block (F 128 + H 64)

# ---- tuning knobs ----
GROUP_BLOCKS = (1, 2, 3, 2)  # blocks per Exp group
ZBUFS = 2
WBUFS = 3
OBUFS = 2

# par layout: per block [LF 8x128 | LH 8x128], then basisF 8x128, basisH 8x64
PAR_BLK = 2 * NO
PAR_BASF = B * PAR_BLK
PAR_BASH = PAR_BASF + NO
PAR_COLS = PAR_BASH + NO // 2


def _build_program(has_bias, group_blocks=None):
    nc = bacc.Bacc("TRN2", target_bir_lowering=False, debug=False,
                   num_devices=NCORES)

    xt_d = nc.dram_tensor("xt", [128, 2 * B * BATCH], bf16,
                          kind="ExternalInput").ap()
    par_d = nc.dram_tensor("par", [NBASIS, PAR_COLS], bf16,
                           kind="ExternalInput").ap()
    bias_d = nc.dram_tensor("biasv", [128, B], f32,
                            kind="ExternalInput").ap()
    out_d = nc.dram_tensor("out", [128, B * BATCH], f32,
                           kind="ExternalOutput").ap()

    gb = list(group_blocks or GROUP_BLOCKS)
    assert sum(gb) == B
    gs = [sum(gb[:i]) for i in range(len(gb) + 1)]  # block offsets
    NG = len(gb)

    with tile.TileContext(nc) as tc, ExitStack() as ctx:
        cpool = ctx.enter_context(tc.tile_pool(name="const", bufs=1))
        wpool = ctx.enter_context(tc.tile_pool(name="wts", bufs=WBUFS))
        spool = ctx.enter_context(tc.tile_pool(name="stage", bufs=NG))
        zpool = ctx.enter_context(tc.tile_pool(name="zq", bufs=ZBUFS,
                                               space="PSUM"))
        opool = ctx.enter_context(tc.tile_pool(name="acc", bufs=OBUFS,
                                               space="PSUM"))

        t_par_t = cpool.tile([NBASIS, PAR_COLS], bf16, tag="par")
        t_par = t_par_t[:]
        nc.sync.dma_start(t_par, par_d)
        t_xt_t = cpool.tile([128, 2 * B * BATCH], bf16, tag="xt")
        t_xt = t_xt_t[:]
        nc.sync.dma_start(t_xt, xt_d)
        t_bias_t = cpool.tile([128, B], f32, tag="bias")
        t_bias = t_bias_t[:]
        if has_bias:
            nc.sync.dma_start(t_bias, bias_d)

        basisF = t_par[:, PAR_BASF:PAR_BASF + NO]
        basisH = t_par[:, PAR_BASH:PAR_BASH + NO // 2]

        def z_group(g):
            zp = zpool.tile([128, gb[g] * WCOLS], f32, tag="z")
            for jl in range(gb[g]):
                j = gs[g] + jl
                base = jl * WCOLS
                nc.tensor.matmul(zp[:, base:base + NO],
                                 t_par[:, j * PAR_BLK:j * PAR_BLK + NO],
                                 basisF, start=True, stop=True)
                nc.tensor.matmul(zp[:, base + NO:base + WCOLS],
                                 t_par[:, j * PAR_BLK + NO:(j + 1) * PAR_BLK],
                                 basisH, start=True, stop=True)
            return zp

        def exp_group(g, zp):
            wt = wpool.tile([128, gb[g] * WCOLS], bf16, tag="w")
            nc.scalar.activation(wt[:], zp[:], AF.Exp, bias=0.0, scale=-1.0)
            return wt

        def mm_group(g, wt):
            og = opool.tile([128, gb[g] * BATCH], f32, tag="og")
            for jl in range(gb[g]):
                j = gs[g] + jl
                base = jl * WCOLS
                ob = jl * BATCH
                xf = t_xt[:, 2 * j * BATCH:(2 * j + 1) * BATCH]
                xh = t_xt[:, (2 * j + 1) * BATCH:(2 * j + 2) * BATCH]
                nc.tensor.matmul(og[:, ob:ob + BATCH],
                                 wt[:, base:base + NO], xf,
                                 start=True, stop=False)
                nc.tensor.matmul(og[0:64, ob:ob + BATCH],
                                 wt[0:64, base + NO:base + WCOLS],
                                 xh[0:64, :], start=False, stop=True)
                nc.tensor.matmul(og[64:128, ob:ob + BATCH],
                                 wt[64:128, base + NO:base + WCOLS],
                                 xh[64:128, :], start=False, stop=True)
            return og

        def relu_dma_group(g, og):
            j0, j1 = gs[g], gs[g + 1]
            sg_own = spool.tile([128, (j1 - j0) * BATCH], f32, tag="sg")
            sg = sg_own[:]
            if has_bias:
                for j in range(j0, j1):
                    jl = j - j0
                    nc.vector.tensor_scalar(sg[:, jl * BATCH:(jl + 1) * BATCH],
                                            og[:, jl * BATCH:(jl + 1) * BATCH],
                                            t_bias[:, j:j + 1], 0.0,
                                            ALU.add, ALU.max)
            else:
                nc.vector.tensor_scalar_max(sg, og[:], 0.0)
            nc.sync.dma_start(out_d[:, j0 * BATCH:j1 * BATCH], sg)

        # Interleave: z groups keep ACT fed ahead of the out-matmuls.
        zps = [None] * NG
        zps[0] = z_group(0)
        if NG > 1:
            zps[1] = z_group(1)
        for g in range(NG):
            wt = exp_group(g, zps[g])
            if g + 2 < NG:
                zps[g + 2] = z_group(g + 2)
            og = mm_group(g, wt)
            relu_dma_group(g, og)

    nc.compile()
    return nc


_PROG_CACHE = {}


def _hilo(a):
    hi = a.astype(BF).astype(np.float64)
    lo = (a - hi).astype(BF)
    return hi.astype(BF), lo


def _coeffs(sc, v):
    """lhs rows pairing with basis {f2h, f2h, f2l, f, f, 1, 1, 0}."""
    A = sc * sc
    Bc = -2.0 * sc * v
    C = v * v
    Ah, Al = _hilo(A)
    Bh, Bl = _hilo(Bc)
    Ch, Cl = _hilo(C)
    return np.stack([Ah, Al, Ah, Bh, Bl, Ch, Cl,
                     np.zeros_like(Ah)], axis=-2)  # [..., 8, n]


def _basis(n):
    """bf16-exact split basis {f2h, f2h, f2l, f, f, 1, 1, 0} for
    f_c = arange(n) - n/2."""
    fc = np.arange(n, dtype=np.float64) - n / 2.0
    f2 = fc * fc
    f2h = f2.astype(BF).astype(np.float64)
    f2l = (f2 - f2h).astype(BF)
    return np.stack([f2h.astype(BF), f2h.astype(BF), f2l,
                     fc.astype(BF), fc.astype(BF),
                     np.ones(n, BF), np.ones(n, BF), np.zeros(n, BF)])


def _prepare(x, mu, sigma, amplitude, bias):
    """Host-side packing: sort by mu, pick per-block 256-row bands, split
    central/outer rows, build hi/lo quadratic coefficients and the
    SBUF-mirrored input maps."""
    mu_f = np.asarray(mu, dtype=np.float64).ravel()
    sg_f = np.asarray(sigma, dtype=np.float64).ravel()
    am_f = np.asarray(amplitude, dtype=np.float64).ravel()
    perm = np.argsort(mu_f, kind="stable")
    mus = mu_f[perm]
    sgs = sg_f[perm]
    ams = am_f[perm]
    xp = np.ascontiguousarray(np.asarray(x, dtype=np.float32)[:, perm])
    if not np.allclose(ams, 1.0):
        xp = xp * ams[None, :].astype(np.float32)
    x_bf = xp.astype(BF)

    nblk = NCORES * B
    centers = np.arange(nblk, dtype=np.float64) * NO + NO / 2.0
    pos = np.searchsorted(mus, centers)                      # rows below c
    # F = the 128 rows centered (by sorted position) on the block center;
    # HL/HR = the 64 rows immediately left/right of F. Rows off the array
    # ends are padding (weight forced to 0). This keeps every H row's
    # in-block Gaussian support inside its 64-col half-window.
    i0 = pos - 64                                            # F start
    ridx = np.concatenate([
        i0[:, None] + np.arange(128)[None, :],               # F
        i0[:, None] - 64 + np.arange(64)[None, :],           # HL
        i0[:, None] + 128 + np.arange(64)[None, :],          # HR
    ], axis=1)                                               # [nblk, 256]
    valid = (ridx >= 0) & (ridx < IN_F)
    ridx = np.clip(ridx, 0, IN_F - 1)
    sc = 1.0 / (np.sqrt(2.0) * np.maximum(sgs[ridx], 1e-30))  # [nblk, 256]
    # per-partition window centers: F window is o in [c-64, c+64) with
    # f_c = f-64; the H window is o in [c-64, c) for left rows (center
    # c-32) and [c, c+64) for right rows (center c+32), f_c = f-32.
    cent = np.empty((nblk, KB), dtype=np.float64)
    cent[:, 0:128] = centers[:, None]
    cent[:, 128:192] = centers[:, None] - 32.0
    cent[:, 192:256] = centers[:, None] + 32.0
    v = sc * (mus[ridx] - cent)
    sc = np.where(valid, sc, 0.0)
    v = np.where(valid, v, 10.0)                             # z=100 -> W=0
    lhs = _coeffs(sc, v)                                     # [nblk, 8, 256]

    bias_v = np.asarray(bias, dtype=np.float32).ravel()
    has_bias = bool(np.any(bias_v != 0.0))

    xg = x_bf[:, ridx]                                       # [64, nblk, 256]
    xg = np.where(valid[None, :, :], xg, np.zeros((), BF))

    in_maps = []
    for c in range(NCORES):
        blk = slice(c * B, (c + 1) * B)
        par = np.empty((NBASIS, PAR_COLS), dtype=BF)
        par[:, :PAR_BASF] = lhs[blk].transpose(1, 0, 2).reshape(NBASIS,
                                                                B * KB)
        par[:, PAR_BASF:PAR_BASF + NO] = _basis(NO)
        par[:, PAR_BASH:] = _basis(NO // 2)
        # xt: per block two chunks [128, 64]: F rows, then H rows (L|R)
        xc = xg[:, blk].reshape(BATCH, B, 2, 128)            # [64, 8, 2, 128]
        xt = np.ascontiguousarray(
            xc.transpose(3, 1, 2, 0).reshape(128, 2 * B * BATCH))
        bm = np.ascontiguousarray(
            bias_v[c * PER_CORE:(c + 1) * PER_CORE].reshape(B, NO).T)
        in_maps.append({"xt": xt, "par": par, "biasv": bm})
    return in_maps, has_bias


def kernel(x, mu, sigma, amplitude, bias, _trace=False):
    in_maps, has_bias = _prepare(x, mu, sigma, amplitude, bias)
    if has_bias not in _PROG_CACHE:
        _PROG_CACHE[has_bias] = _build_program(has_bias)
    nc = _PROG_CACHE[has_bias]
    res = bass_utils.run_bass_kernel_spmd(nc, in_maps, list(range(NCORES)),
                                          trace=_trace)
    out = np.empty((BATCH, OUT_F), dtype=np.float32)
    for c in range(NCORES):
        # [128, B*BATCH] -> out[b, c*1024 + j*128 + p]
        arr = res.results[c]["out"].reshape(128, B, BATCH)
        out[:, c * PER_CORE:(c + 1) * PER_CORE] = \
            arr.transpose(2, 1, 0).reshape(BATCH, PER_CORE)
    if _trace:
        kernel._last = res
    return out


# revision 23
# speedup vs baseline: 1.0845x; 1.0305x over previous
"""GaussianFC Trainium2 kernel.

out = relu(x @ W + bias),  W[i, o] = amp[i] * exp(-(o - mu[i])^2 / (2 sigma[i]^2))

Strategy (8 NeuronCores, out_features sharded, 1024 cols/core):
- Banded weights: sigma ~ 10 makes W effectively zero outside |o - mu| ~ 45.
  Host sorts rows by mu; each 128-col output block reads the 256 nearest
  (in mu) input rows at arbitrary unaligned offsets.
- z = (sc*(o - mu))^2 is quadratic in the column index, so z tiles are
  rank-3 outer products: K=8 bf16 matmuls on PE against a fixed basis
  {f^2_hi, f^2_lo, f, 1} with hi/lo-split per-partition coefficients
  (z error ~5e-3). No DVE/ACT synthesis work at all.
- Each block's 256-row band is split: the central 128 rows get the full
  128-col window; the 128 outer rows (64 left + 64 right) share one
  64-col window (left rows use the left-half columns, right rows the
  right half — the per-partition quadratic centers absorb the shift).
  That cuts synthesized area from 256 to 192 cols/block. The outer-row
  matmuls run on partition ranges [0:64) / [64:128) with separate PSUM
  stop flags.
- W = Exp(-z) runs as one parameter-free ACT op per group (PSUM -> SBUF
  bf16); the serial Exp chain is the kernel's ceiling.
- Main matmuls keep W stationary (lhs) and stream x (64 moving rows,
  bf16): out^T[o, b] accumulates in PSUM; relu (+bias) on DVE; outputs
  leave in SBUF-mirrored DRAM layout, host undoes the transpose.
"""
import numpy as np
from contextlib import ExitStack

import ml_dtypes

import concourse.bacc as bacc
import concourse.bass as bass
import concourse.mybir as mybir
import concourse.tile as tile
from concourse import bass_utils

f32 = mybir.dt.float32
bf16 = mybir.dt.bfloat16
AF = mybir.ActivationFunctionType
ALU = mybir.AluOpType
BF = ml_dtypes.bfloat16

NCORES = 8
BATCH = 64
IN_F = 8192
OUT_F = 8192
PER_CORE = OUT_F // NCORES  # 1024
NO = 128                    # output cols per block
B = PER_CORE // NO          # 8 blocks per core
KB = 256                    # band rows per block (128 core + 128 outer)
NBASIS = 8                  # quadratic basis rows (7 used + 1 pad)
WCOLS = NO + NO // 2        # 192 synthesized cols per block (F 128 + H 64)

# ---- tuning knobs ----
GROUP_BLOCKS = (2, 3, 3)     # blocks per Exp group
DMA_BUNDLES = ((0, 1), (2,))  # exp-groups sharing one output DMA
ZBUFS = 2
WBUFS = 3
OBUFS = 2

# par layout: per block [LF 8x128 | LH 8x128], then basisF 8x128, basisH 8x64
PAR_BLK = 2 * NO
PAR_BASF = B * PAR_BLK
PAR_BASH = PAR_BASF + NO
PAR_COLS = PAR_BASH + NO // 2


def _build_program(has_bias, group_blocks=None, dma_bundles=None):
    nc = bacc.Bacc("TRN2", target_bir_lowering=False, debug=False,
                   num_devices=NCORES)

    xt_d = nc.dram_tensor("xt", [128, 2 * B * BATCH], bf16,
                          kind="ExternalInput").ap()
    par_d = nc.dram_tensor("par", [NBASIS, PAR_COLS], bf16,
                           kind="ExternalInput").ap()
    bias_d = nc.dram_tensor("biasv", [128, B], f32,
                            kind="ExternalInput").ap()
    out_d = nc.dram_tensor("out", [128, B * BATCH], f32,
                           kind="ExternalOutput").ap()

    gb = list(group_blocks or GROUP_BLOCKS)
    assert sum(gb) == B
    gs = [sum(gb[:i]) for i in range(len(gb) + 1)]  # block offsets
    NG = len(gb)
    bundles = [list(bn) for bn in (dma_bundles or DMA_BUNDLES)]
    grp_bundle = {}
    for bi, bn in enumerate(bundles):
        for g in bn:
            grp_bundle[g] = bi

    with tile.TileContext(nc) as tc, ExitStack() as ctx:
        cpool = ctx.enter_context(tc.tile_pool(name="const", bufs=1))
        wpool = ctx.enter_context(tc.tile_pool(name="wts", bufs=WBUFS))
        spool = ctx.enter_context(tc.tile_pool(name="stage", bufs=NG))
        zpool = ctx.enter_context(tc.tile_pool(name="zq", bufs=ZBUFS,
                                               space="PSUM"))
        opool = ctx.enter_context(tc.tile_pool(name="acc", bufs=OBUFS,
                                               space="PSUM"))

        t_par_t = cpool.tile([NBASIS, PAR_COLS], bf16, tag="par")
        t_par = t_par_t[:]
        nc.sync.dma_start(t_par, par_d)
        t_xt_t = cpool.tile([128, 2 * B * BATCH], bf16, tag="xt")
        t_xt = t_xt_t[:]
        nc.sync.dma_start(t_xt, xt_d)
        t_bias_t = cpool.tile([128, B], f32, tag="bias")
        t_bias = t_bias_t[:]
        if has_bias:
            nc.sync.dma_start(t_bias, bias_d)

        basisF = t_par[:, PAR_BASF:PAR_BASF + NO]
        basisH = t_par[:, PAR_BASH:PAR_BASH + NO // 2]

        def z_group(g):
            zp = zpool.tile([128, gb[g] * WCOLS], f32, tag="z")
            for jl in range(gb[g]):
                j = gs[g] + jl
                base = jl * WCOLS
                nc.tensor.matmul(zp[:, base:base + NO],
                                 t_par[:, j * PAR_BLK:j * PAR_BLK + NO],
                                 basisF, start=True, stop=True)
                nc.tensor.matmul(zp[:, base + NO:base + WCOLS],
                                 t_par[:, j * PAR_BLK + NO:(j + 1) * PAR_BLK],
                                 basisH, start=True, stop=True)
            return zp

        def exp_group(g, zp):
            wt = wpool.tile([128, gb[g] * WCOLS], bf16, tag="w")
            nc.scalar.activation(wt[:], zp[:], AF.Exp, bias=0.0, scale=-1.0)
            return wt

        def mm_group(g, wt):
            og = opool.tile([128, gb[g] * BATCH], f32, tag="og")
            for jl in range(gb[g]):
                j = gs[g] + jl
                base = jl * WCOLS
                ob = jl * BATCH
                xf = t_xt[:, 2 * j * BATCH:(2 * j + 1) * BATCH]
                xh = t_xt[:, (2 * j + 1) * BATCH:(2 * j + 2) * BATCH]
                nc.tensor.matmul(og[:, ob:ob + BATCH],
                                 wt[:, base:base + NO], xf,
                                 start=True, stop=False)
                nc.tensor.matmul(og[0:64, ob:ob + BATCH],
                                 wt[0:64, base + NO:base + WCOLS],
                                 xh[0:64, :], start=False, stop=True)
                nc.tensor.matmul(og[64:128, ob:ob + BATCH],
                                 wt[64:128, base + NO:base + WCOLS],
                                 xh[64:128, :], start=False, stop=True)
            return og

        bundle_tiles = {}

        def relu_dma_group(g, og):
            j0, j1 = gs[g], gs[g + 1]
            bi = grp_bundle[g]
            bg0, bg1 = bundles[bi][0], bundles[bi][-1]
            bj0, bj1 = gs[bg0], gs[bg1 + 1]
            if bi not in bundle_tiles:
                sg_bundle = spool.tile([128, (bj1 - bj0) * BATCH], f32,
                                       tag="sg")
                bundle_tiles[bi] = sg_bundle
            sg = bundle_tiles[bi][:, (j0 - bj0) * BATCH:(j1 - bj0) * BATCH]
            if has_bias:
                for j in range(j0, j1):
                    jl = j - j0
                    nc.vector.tensor_scalar(sg[:, jl * BATCH:(jl + 1) * BATCH],
                                            og[:, jl * BATCH:(jl + 1) * BATCH],
                                            t_bias[:, j:j + 1], 0.0,
                                            ALU.add, ALU.max)
            else:
                nc.vector.tensor_scalar_max(sg, og[:], 0.0)
            if g == bg1:
                nc.sync.dma_start(out_d[:, bj0 * BATCH:bj1 * BATCH],
                                  bundle_tiles[bi][:])

        # Interleave: z groups keep ACT fed ahead of the out-matmuls.
        zps = [None] * NG
        zps[0] = z_group(0)
        if NG > 1:
            zps[1] = z_group(1)
        for g in range(NG):
            wt = exp_group(g, zps[g])
            if g + 2 < NG:
                zps[g + 2] = z_group(g + 2)
            og = mm_group(g, wt)
            relu_dma_group(g, og)

    nc.compile()
    return nc


_PROG_CACHE = {}


def _hilo(a):
    hi = a.astype(BF).astype(np.float64)
    lo = (a - hi).astype(BF)
    return hi.astype(BF), lo


def _coeffs(sc, v):
    """lhs rows pairing with basis {f2h, f2h, f2l, f, f, 1, 1, 0}."""
    A = sc * sc
    Bc = -2.0 * sc * v
    C = v * v
    Ah, Al = _hilo(A)
    Bh, Bl = _hilo(Bc)
    Ch, Cl = _hilo(C)
    return np.stack([Ah, Al, Ah, Bh, Bl, Ch, Cl,
                     np.zeros_like(Ah)], axis=-2)  # [..., 8, n]


def _basis(n):
    """bf16-exact split basis {f2h, f2h, f2l, f, f, 1, 1, 0} for
    f_c = arange(n) - n/2."""
    fc = np.arange(n, dtype=np.float64) - n / 2.0
    f2 = fc * fc
    f2h = f2.astype(BF).astype(np.float64)
    f2l = (f2 - f2h).astype(BF)
    return np.stack([f2h.astype(BF), f2h.astype(BF), f2l,
                     fc.astype(BF), fc.astype(BF),
                     np.ones(n, BF), np.ones(n, BF), np.zeros(n, BF)])


def _prepare(x, mu, sigma, amplitude, bias):
    """Host-side packing: sort by mu, pick per-block 256-row bands, split
    central/outer rows, build hi/lo quadratic coefficients and the
    SBUF-mirrored input maps."""
    mu_f = np.asarray(mu, dtype=np.float64).ravel()
    sg_f = np.asarray(sigma, dtype=np.float64).ravel()
    am_f = np.asarray(amplitude, dtype=np.float64).ravel()
    perm = np.argsort(mu_f, kind="stable")
    mus = mu_f[perm]
    sgs = sg_f[perm]
    ams = am_f[perm]
    xp = np.ascontiguousarray(np.asarray(x, dtype=np.float32)[:, perm])
    if not np.allclose(ams, 1.0):
        xp = xp * ams[None, :].astype(np.float32)
    x_bf = xp.astype(BF)

    nblk = NCORES * B
    centers = np.arange(nblk, dtype=np.float64) * NO + NO / 2.0
    pos = np.searchsorted(mus, centers)                      # rows below c
    # F = the 128 rows centered (by sorted position) on the block center;
    # HL/HR = the 64 rows immediately left/right of F. Rows off the array
    # ends are padding (weight forced to 0). This keeps every H row's
    # in-block Gaussian support inside its 64-col half-window.
    i0 = pos - 64                                            # F start
    ridx = np.concatenate([
        i0[:, None] + np.arange(128)[None, :],               # F
        i0[:, None] - 64 + np.arange(64)[None, :],           # HL
        i0[:, None] + 128 + np.arange(64)[None, :],          # HR
    ], axis=1)                                               # [nblk, 256]
    valid = (ridx >= 0) & (ridx < IN_F)
    ridx = np.clip(ridx, 0, IN_F - 1)
    sc = 1.0 / (np.sqrt(2.0) * np.maximum(sgs[ridx], 1e-30))  # [nblk, 256]
    # per-partition window centers: F window is o in [c-64, c+64) with
    # f_c = f-64; the H window is o in [c-64, c) for left rows (center
    # c-32) and [c, c+64) for right rows (center c+32), f_c = f-32.
    cent = np.empty((nblk, KB), dtype=np.float64)
    cent[:, 0:128] = centers[:, None]
    cent[:, 128:192] = centers[:, None] - 32.0
    cent[:, 192:256] = centers[:, None] + 32.0
    v = sc * (mus[ridx] - cent)
    sc = np.where(valid, sc, 0.0)
    v = np.where(valid, v, 10.0)                             # z=100 -> W=0
    lhs = _coeffs(sc, v)                                     # [nblk, 8, 256]

    bias_v = np.asarray(bias, dtype=np.float32).ravel()
    has_bias = bool(np.any(bias_v != 0.0))

    xg = x_bf[:, ridx]                                       # [64, nblk, 256]
    xg = np.where(valid[None, :, :], xg, np.zeros((), BF))

    in_maps = []
    for c in range(NCORES):
        blk = slice(c * B, (c + 1) * B)
        par = np.empty((NBASIS, PAR_COLS), dtype=BF)
        par[:, :PAR_BASF] = lhs[blk].transpose(1, 0, 2).reshape(NBASIS,
                                                                B * KB)
        par[:, PAR_BASF:PAR_BASF + NO] = _basis(NO)
        par[:, PAR_BASH:] = _basis(NO // 2)
        # xt: per block two chunks [128, 64]: F rows, then H rows (L|R)
        xc = xg[:, blk].reshape(BATCH, B, 2, 128)            # [64, 8, 2, 128]
        xt = np.ascontiguousarray(
            xc.transpose(3, 1, 2, 0).reshape(128, 2 * B * BATCH))
        bm = np.ascontiguousarray(
            bias_v[c * PER_CORE:(c + 1) * PER_CORE].reshape(B, NO).T)
        in_maps.append({"xt": xt, "par": par, "biasv": bm})
    return in_maps, has_bias


def kernel(x, mu, sigma, amplitude, bias, _trace=False):
    in_maps, has_bias = _prepare(x, mu, sigma, amplitude, bias)
    if has_bias not in _PROG_CACHE:
        _PROG_CACHE[has_bias] = _build_program(has_bias)
    nc = _PROG_CACHE[has_bias]
    res = bass_utils.run_bass_kernel_spmd(nc, in_maps, list(range(NCORES)),
                                          trace=_trace)
    out = np.empty((BATCH, OUT_F), dtype=np.float32)
    for c in range(NCORES):
        # [128, B*BATCH] -> out[b, c*1024 + j*128 + p]
        arr = res.results[c]["out"].reshape(128, B, BATCH)
        out[:, c * PER_CORE:(c + 1) * PER_CORE] = \
            arr.transpose(2, 1, 0).reshape(BATCH, PER_CORE)
    if _trace:
        kernel._last = res
    return out
